# revision 21
# baseline (speedup 1.0000x reference)
"""Causal single-head attention on 8 TRN2 NeuronCores, data-parallel over batch.

Per core (one batch element): x [T=2048, C=1024], weights [C, H=128].
  q = x@Wq + bq ; k = x@Wk + bk ; v = x@Wv + bv
  out = softmax(mask(q k^T / sqrt(H))) @ v

Layout strategy (no on-device transposes anywhere):
  - host packs x^T bf16 into per-chunk contiguous tiles: xh[j*128+p,
    o*512+t'] = x[j*512+t', o*128+p], so each 512-col chunk loads with ONE
    dma_start of 128 contiguous 8KB descriptors (minimal issue + ring cost)
  - weights packed the same way: wh[p, o*384+c] = wqkv[o*128+p, c]
  - projections contract C on partitions: qT, kT [H, T], v [T, H]
  - scores computed transposed, S'[s, t] = k q^T, via stationary kT[:, s128]
  - softmax sums via a ones-column appended to v: the PV matmul per t-chunk
    yields both sum_s P'[s,t] v[s,h] and sum_s P'[s,t]
  - causal: blocks above the diagonal are skipped, diagonal s-tiles compute
    only the valid t' range, one [128,128] triangular mask on the mixed chunk
  - projection work for chunk j+1 is emitted interleaved into attention
    block j so the PE never stalls on the softmax exp
  - matmul inputs bf16 (fp32 PSUM accumulation); output + biases fp32
  - PSUM->SBUF projection copies on DVE (ACT is exp-bound in late blocks)
  - narrow 128-wide warmup matmuls bridge the input-DMA latency so HAM is
    at K=8/8 when real work starts
"""

import numpy as np
import ml_dtypes

import concourse.bass as bass
import concourse.mybir as mybir
import concourse.tile as tile
from concourse.bass_utils import run_bass_kernel_spmd

F32 = mybir.dt.float32
BF16 = mybir.dt.bfloat16
AF = mybir.ActivationFunctionType

B, T, C, H = 8, 2048, 1024, 128
P = 128
CT = C // P        # 8 contraction tiles
TBLK = 512         # t-block / projection chunk width
NBLK = T // TBLK   # 4
NST = T // P       # 16 s-tiles
SCALE = 1.0 / float(np.sqrt(H))
XW = CT * TBLK     # 4096 elems per partition per chunk

N_CORES = 8
N_WARM = 19
WARMW = 256


def _split_multiwaits(nc, max_waits=1):
    """walrus in this image rejects >1 sem wait on one instruction; hoist
    extras onto single-wait NOPs placed just before on the same engine."""
    n_new = 0
    for fn in nc.m.functions:
        for bb in fn.blocks:
            new_insts = []
            for ins in bb.instructions:
                si = ins.sync_info
                if si is not None and si.on_wait and len(si.on_wait) > max_waits:
                    waits = list(si.on_wait)
                    for w in waits[:-max_waits]:
                        n_new += 1
                        new_insts.append(
                            mybir.InstNoOp(
                                name=f"I-waitsplit-{n_new}",
                                engine=ins.engine,
                                ins=[],
                                outs=[],
                                sync_info=mybir.SyncInfo(on_wait=[w], on_update=[]),
                            )
                        )
                    ins.sync_info = mybir.SyncInfo(
                        on_wait=waits[-max_waits:],
                        on_update=list(si.on_update or []),
                    )
                new_insts.append(ins)
            bb.instructions = new_insts
    return n_new


def _trim_tail(nc, mode=1):
    """Teardown trim. The NEFF is executed once per load, so the semaphore /
    DMA-ring reset block only exists to leave state clean for a re-execute
    that never happens; NRT expands the ring reset into a multi-microsecond
    per-ring storm at the end of every engine queue.

    mode 0: drop everything after the last InstISA (post-clear barrier) --
            the previous baseline behavior.
    mode 1: additionally drop the Pool dma_reset / sem range-clear and the
            whole inter-engine barrier: keep only the SP global drain (whose
            sem waits gate on every DMA completion, including outputs).
    """
    for fn in nc.m.functions:
        for bb in fn.blocks:
            if not bb.name.endswith("_build_end"):
                continue
            if mode == 0:
                last_isa = None
                for i, ins in enumerate(bb.instructions):
                    if type(ins).__name__ == "InstISA":
                        last_isa = i
                if last_isa is not None:
                    bb.instructions = bb.instructions[: last_isa + 1]
            elif mode == 1:
                # keep leading SP NoOps (split waits) + the first SP drain
                keep = []
                for ins in bb.instructions:
                    tn = type(ins).__name__
                    eng = str(ins.engine)
                    if tn == "InstNoOp":
                        keep.append(ins)
                        continue
                    if tn == "InstDrain" and eng.endswith("SP"):
                        keep.append(ins)
                        break
                    break
                bb.instructions = keep


def _build(split=True, with_bias=False, trim=1, n_warm=N_WARM):
    nc = bass.Bass()
    xh = nc.declare_dram_parameter("xh", [NBLK * P, XW], BF16, isOutput=False)
    wh = nc.declare_dram_parameter("wh", [P, CT * 3 * H], BF16, isOutput=False)
    if with_bias:
        bqk = nc.declare_dram_parameter("bqk", [H, 2], F32, isOutput=False)
        bv = nc.declare_dram_parameter("bv", [H], F32, isOutput=False)
    out = nc.declare_dram_parameter("out", [T, H], F32, isOutput=True)

    with (
        tile.TileContext(nc) as tc,
        tc.tile_pool(name="singles", bufs=1) as singles,
        tc.tile_pool(name="psbp", bufs=3) as psbp,
        tc.tile_pool(name="osbp", bufs=4) as osbp,
        tc.tile_pool(name="rsbp", bufs=4) as rsbp,
        tc.tile_pool(name="ps_prj", bufs=2, space="PSUM") as ps_prj,
        tc.tile_pool(name="ps_s", bufs=2, space="PSUM") as ps_s,  # [P,2,TBLK] pairs
        tc.tile_pool(name="ps_o", bufs=1, space="PSUM") as ps_o,
    ):
        # ---- input DMAs first, split across BOTH HWDGE rings (SP + ACT)
        # so they stream concurrently at the full HBM rate, ordered by when
        # the pipeline needs each piece. Chunks are halved by c-tile so
        # projections can start on partial data and both rings stay busy. ----
        w_bf = singles.tile([P, 3 * CT * H], BF16)   # proj-major: q | k | v
        x_sb = singles.tile([P, NBLK * XW], BF16)
        CH = CT * H  # 1024 cols per projection

        def xdma(eng, j, half):
            lo = j * XW + half * (XW // 2)
            src0 = (j * P, half * (XW // 2))
            eng.dma_start(
                x_sb[:, lo : lo + XW // 2],
                xh[src0[0] : src0[0] + P, src0[1] : src0[1] + XW // 2],
            )

        QW = XW // 4  # quarter chunk (two c-tiles)

        def xqdma(eng, j, quarter):
            lo = j * XW + quarter * QW
            eng.dma_start(
                x_sb[:, lo : lo + QW],
                xh[j * P : (j + 1) * P, quarter * QW : (quarter + 1) * QW],
            )

        nc.sync.dma_start(w_bf[:, 0:CH], wh[:, 0:CH])            # wq
        xqdma(nc.scalar, 0, 1)                                   # c0 o2-3
        xqdma(nc.sync, 0, 0)                                     # c0 o0-1
        nc.scalar.dma_start(w_bf[:, CH : 2 * CH], wh[:, CH : 2 * CH])  # wk
        xqdma(nc.sync, 0, 2)                                     # c0 o4-5
        xqdma(nc.scalar, 0, 3)                                   # c0 o6-7
        nc.sync.dma_start(w_bf[:, 2 * CH : 3 * CH], wh[:, 2 * CH : 3 * CH])  # wv
        for j in range(1, NBLK):
            xdma(nc.sync, j, 0)
            xdma(nc.scalar, j, 1)

        def ws(p_idx, o):
            base = p_idx * CH + o * H
            return w_bf[:, base : base + H]

        def xs(j, o):
            base = j * XW + o * TBLK
            return x_sb[:, base : base + TBLK]

        def xs128(j, o, m4):
            base = j * XW + o * TBLK + m4 * P
            return x_sb[:, base : base + P]

        # ---- PE warmup: release the HAM clock gate while DMAs land; sized
        # to end just as the first projection's inputs arrive so the PE never
        # idles long enough to re-throttle ----
        warm = singles.tile([P, WARMW], BF16)
        nc.gpsimd.memset(warm[:], 0.0)
        ps_warm = ps_s.tile([P, 2, TBLK], F32, tag="ps", name="ps_warm")
        for _ in range(n_warm):
            nc.tensor.matmul(
                ps_warm[:, 0, 0:WARMW], warm[:, 0:P], warm[:], start=True, stop=True
            )

        if with_bias:
            bqk_sb = singles.tile([P, 2], F32)
            nc.gpsimd.dma_start(bqk_sb[:], bqk[:, :])
            bv_rep = singles.tile([P, H], F32)
            bv_ap = bv[:]
            nc.gpsimd.dma_start(
                bv_rep[:],
                bass.AP(
                    tensor=bv_ap.tensor, offset=bv_ap.offset, ap=[[0, P], [1, H]]
                ),
            )

        # triangular mask [128,128]: mask[i, t''] = 1.0 if t'' >= i else 0.0
        mask = singles.tile([P, P], BF16)
        nc.gpsimd.memset(mask[:], 1.0)
        nc.gpsimd.affine_select(
            out=mask[:],
            in_=mask[:],
            compare_op=mybir.AluOpType.is_ge,
            fill=0.0,
            base=0,
            pattern=[[1, P]],
            channel_multiplier=-1,
        )

        qT_sb = singles.tile([P, T], BF16)   # [h, t]
        kT_sb = singles.tile([P, T], BF16)   # [h, t]
        v_sb = singles.tile([P, NST, 132], BF16)  # [s128, s-tile, h | ones]
        nc.gpsimd.memset(v_sb[:], 1.0)

        def gq(j):
            """q-projection of chunk j (9 units); must finish before block j."""
            t0 = j * TBLK
            pqk = ps_prj.tile([P, TBLK], F32, tag="prj", name="pq")
            for o in range(CT):
                nc.tensor.matmul(
                    pqk[:], ws(0, o), xs(j, o),
                    start=(o == 0), stop=(o == CT - 1),
                )
                yield
            if with_bias:
                nc.scalar.activation(
                    qT_sb[:, t0 : t0 + TBLK], pqk[:], AF.Identity,
                    bias=bqk_sb[:, 0:1],
                )
            else:
                nc.scalar.activation(qT_sb[:, t0 : t0 + TBLK], pqk[:], AF.Copy)
            yield

        def gk(j):
            """k-projection of chunk j (9 units); needed by scores m >= 4j."""
            t0 = j * TBLK
            pqk = ps_prj.tile([P, TBLK], F32, tag="prj", name="pk")
            for o in range(CT):
                nc.tensor.matmul(
                    pqk[:], ws(1, o), xs(j, o),
                    start=(o == 0), stop=(o == CT - 1),
                )
                yield
            if with_bias:
                nc.scalar.activation(
                    kT_sb[:, t0 : t0 + TBLK], pqk[:], AF.Identity,
                    bias=bqk_sb[:, 1:2],
                )
            else:
                nc.scalar.activation(kT_sb[:, t0 : t0 + TBLK], pqk[:], AF.Copy)
            yield

        def gv(j):
            """v-projection of chunk j (9 units); needed by PV m >= 4j."""
            pvv = ps_prj.tile([P, 4, H], F32, tag="prj", name="pv")
            for m4 in range(4):
                for o in range(CT):
                    nc.tensor.matmul(
                        pvv[:, m4, :],
                        xs128(j, o, m4),
                        ws(2, o),
                        start=(o == 0), stop=(o == CT - 1),
                    )
                    if o % 2 == 1:
                        yield
            nc.vector.tensor_copy(v_sb[:, 4 * j : 4 * j + 4, 0:H], pvv[:])
            yield

        # chunk 0 projections up-front (block 0 needs all three immediately)
        for g in (gq(0), gk(0), gv(0)):
            for _ in g:
                pass

        # projection placement: generators interleaved into each block, split
        # by deadline: "early" must drain before iteration m=4j of the block
        # (own-chunk k/v feeding the diagonal tiles), "late" by block end
        # (next chunk's q, and k when it fits). Balances PE load against the
        # exp-bound late blocks.
        early_plan = {}
        late_plan = {0: [gq, gk, gv], 1: [gq, gk, gv], 2: [gq, gk, gv]}

        for j in range(NBLK):
            t0 = j * TBLK
            n_s = 4 * (j + 1)
            early_gens = [g(j) for g in early_plan.get(j, [])]
            late_gens = [g(j + 1) for g in late_plan.get(j, [])]
            early_left = 9 * len(early_gens)
            late_left = 9 * len(late_gens)

            def drain(gens, k):
                done = 0
                while k > 0 and gens:
                    try:
                        next(gens[0])
                        k -= 1
                        done += 1
                    except StopIteration:
                        gens.pop(0)
                return done

            # two po pair-tiles: output columns (0,1) share a PSUM bank and
            # (2,3) the other (the fp32 score pairs use 4 banks, so only 2
            # remain). Only the first matmul into each BANK starts the
            # accumulation group — first_mm clears has_written for the whole
            # bank, and the second column's first matmul (start=False) then
            # overwrites-where-unset, which is exactly "initialize".
            po_pairs = [
                ps_o.tile([P, 2, 132], F32, tag=f"pop{i}", name=f"pop{i}")
                for i in range(2)
            ]

            def po(c):
                return po_pairs[c // 2][:, c % 2, :]

            def pv_mms(m, pb, h):
                r = m - 4 * j
                for c in range(max(r, 0), 4):
                    nc.tensor.matmul(
                        po(c)[:, 0 : H + 1],
                        pb[:, h, c * P : (c + 1) * P],
                        v_sb[:, m, 0 : H + 1],
                        start=(m == 0 and c % 2 == 0),
                        stop=(m == 4 * j + c),
                    )

            def epilogue(c):
                rec = rsbp.tile([P, 1], F32, tag="rec", name="rec")
                nc.vector.reciprocal(rec[:], po(c)[:, H : H + 1])
                o_sb = osbp.tile([P, H], F32, tag="o_sb", name="o_sb")
                nc.vector.tensor_scalar_mul(o_sb[:], po(c)[:, 0:H], rec[:])
                if with_bias:
                    nc.vector.tensor_add(o_sb[:], o_sb[:], bv_rep[:])
                nc.sync.dma_start(out[t0 + c * P : t0 + (c + 1) * P, :], o_sb[:])

            def post_pv(m, pb, h):
                pv_mms(m, pb, h)
                c_done = m - 4 * j
                if c_done in (1, 3):
                    # epilogues read a shared bank; defer to the pair's stop
                    epilogue(c_done - 1)
                    epilogue(c_done)

            prev = None
            for mp in range(0, n_s, 2):
                pp = ps_s.tile([P, 2, TBLK], F32, tag="ps", name="ps")
                pb = psbp.tile([P, 2, TBLK], BF16, tag="p_sb", name="p_sb")
                los = []
                for h in (0, 1):
                    m = mp + h
                    lo = P * max(m - 4 * j, 0)
                    los.append(lo)
                    nc.tensor.matmul(
                        pp[:, h, lo:TBLK],
                        kT_sb[:, m * P : (m + 1) * P],
                        qT_sb[:, t0 + lo : t0 + TBLK],
                        start=True, stop=True,
                    )
                # NOTE: one exp across both halves would halve the ACT
                # per-instruction overhead, but an ACT read spanning two PSUM
                # banks returns garbage on HW (verified) — keep them separate.
                for h in (0, 1):
                    lo = los[h]
                    nc.scalar.activation(
                        pb[:, h, lo:TBLK], pp[:, h, lo:TBLK], AF.Exp,
                        scale=SCALE,
                    )
                for h in (0, 1):
                    m = mp + h
                    if m - 4 * j >= 0:
                        lo = los[h]
                        nc.vector.tensor_mul(
                            pb[:, h, lo : lo + P], pb[:, h, lo : lo + P], mask[:]
                        )
                # previous pair's PV matmuls (their exp has long completed)
                if prev is not None:
                    pm, ppb = prev
                    post_pv(pm, ppb, 0)
                    post_pv(pm + 1, ppb, 1)
                prev = (mp, pb)
                # paced interleave of projection units (after PVs: proj may
                # wait on input DMA, PVs never do)
                if early_left:
                    pairs_to_dl = max((4 * j - mp) // 2, 1)
                    early_left -= drain(early_gens, -(-early_left // pairs_to_dl))
                if late_left:
                    pairs_left = (n_s - mp) // 2
                    late_left -= drain(late_gens, -(-late_left // pairs_left))
            pm, ppb = prev
            post_pv(pm, ppb, 0)
            post_pv(pm + 1, ppb, 1)
            for gens in (early_gens, late_gens):
                while gens:
                    drain(gens, 9)

    if split:
        _trim_tail(nc, mode=trim)
        _split_multiwaits(nc)
    return nc


_NC_CACHE = {}


def _get_nc(with_bias=False):
    key = bool(with_bias)
    if key not in _NC_CACHE:
        _NC_CACHE[key] = _build(with_bias=key)
    return _NC_CACHE[key]


def _prepare_in_maps(batch_x, Wq, bq, Wk, bk, Wv, bv, with_bias):
    # proj-major: wh[p, i*1024 + o*128 + c] = W_i[o*128 + p, c]
    wh = np.ascontiguousarray(
        np.concatenate(
            [
                np.asarray(w)
                .astype(ml_dtypes.bfloat16)
                .reshape(CT, P, H)
                .transpose(1, 0, 2)
                .reshape(P, CT * H)
                for w in (Wq, Wk, Wv)
            ],
            axis=1,
        )
    )
    extra = {}
    if with_bias:
        extra["bqk"] = np.ascontiguousarray(
            np.stack([np.asarray(bq), np.asarray(bk)], axis=1).astype(np.float32)
        )
        extra["bv"] = np.ascontiguousarray(np.asarray(bv).astype(np.float32))
    bx = np.asarray(batch_x)
    in_maps = []
    for i in range(N_CORES):
        # xh[j*128 + p, o*512 + t'] = x[j*512 + t', o*128 + p]
        xT = bx[i].T.astype(ml_dtypes.bfloat16)  # [C, T]
        xhost = np.ascontiguousarray(
            xT.reshape(CT, P, NBLK, TBLK).transpose(2, 1, 0, 3).reshape(
                NBLK * P, CT * TBLK
            )
        )
        in_maps.append({"xh": xhost, "wh": wh, **extra})
    return in_maps


def _needs_bias(bq, bk, bv):
    return bool(
        np.any(np.asarray(bq)) or np.any(np.asarray(bk)) or np.any(np.asarray(bv))
    )


def kernel(batch_x, Wq, bq, Wk, bk, Wv, bv):
    wb = _needs_bias(bq, bk, bv)
    nc = _get_nc(with_bias=wb)
    in_maps = _prepare_in_maps(batch_x, Wq, bq, Wk, bk, Wv, bv, with_bias=wb)
    res = run_bass_kernel_spmd(nc, in_maps, core_ids=list(range(N_CORES)))
    return np.stack([res.results[i]["out"] for i in range(N_CORES)], axis=0)
